# revision 11
# baseline (speedup 1.0000x reference)
"""Trainium2 Bass kernel for nn_LuongAttention.

Reference math (per batch b):
    S   = Dec @ Enc^T          # [T_dec, T_enc]
    Out = S @ Enc              # [T_dec, D]

By associativity:  Out = Dec @ (Enc^T @ Enc) = Dec @ G with G = Enc^T Enc
a [D, D] = [128, 128] Gram matrix.  This removes the [2048, 2048]
intermediate entirely and makes the kernel memory-bound.

Sharding: data-parallel over batch B=8 -> one batch per NeuronCore.

Precision plan (tolerance 2e-2, measured 7.2e-3):
  - enc is fed as fp8 E4M3 (values ~N(0,1), |max| ~5 << 240): halves the
    encoder DMA bytes and enables DoubleRow matmuls (2 row-tiles per
    pass, 0.5 cycles/row - clock-state independent); quantization error
    averages out over the 2048-term Gram contraction.
  - dect stays fp16 (its error multiplies the ~2048 Gram diagonal);
    G cast to fp16 (fp8 G overflows); out fp16, host upcasts.

Schedule (from the deterministic cost model: dma issue ~650ns/ring,
DGE delay 650/784ns, transfers 360GB/s >=512B lines serialized on one
global DMA device, +900ns sem after each transfer; PE: 1.2GHz until
3us of *continuous* execution, then 2.4GHz):
  - sync ring: enc (256KB fp8), dect lo half; scalar ring: dect hi
    half (its descriptors arrive first -> transfers right after enc).
  - Warmup/filler matmuls keep the PE queue busy across the
    load-wait and the G-cast: a PE that goes idle pays a ~0.3-0.9us
    wake-up stall on its next matmul (measured), and continuous
    activity occasionally wins the run-random 1.2->2.4GHz clock flip.
  - Finals 4x512 cols; PSUM->SBUF copies alternate DVE / Scalar-ACT;
    stores alternate sync/scalar rings.
"""

import os
import sys
from contextlib import ExitStack

import numpy as np

for _p in (
    "/opt/trn_rl_repo",
    "/root/.axon_site",
    "/root/.axon_site/_ro/trn_rl_repo",
    "/root/.axon_site/_ro/pypackages",
):
    if os.path.isdir(_p) and _p not in sys.path:
        sys.path.append(_p)

import concourse.bacc as bacc
import concourse.mybir as mybir
import concourse.tile as tile
from concourse.bass_utils import run_bass_kernel_spmd

B, T, D, P = 8, 2048, 128, 128
NT = T // P  # 16 row tiles of 128

# tunables
FINAL_N = 512  # final matmul chunk (= PSUM bank)
W_WARM = 22  # 128-col warmups: keep PE busy until the enc sem (avoids ~0.3-0.9us PE wake-up stall at gram start; occasionally wins the 2.4GHz clock flip)
W_FILL = 10  # 64-col fillers bridging gram -> first final (avoids ~0.9us PE wake-up stall after the G cast; measured -0.5us)
ACT_COPY = True  # alternate PSUM->SBUF copies between DVE and ACT
COPY_SPLIT = ""  # "act" | "": split each copy across 2 engines (measured slower)
SWAP_LANES = True  # last store on sync ring (faster DGE), first copy on ACT


def _build_nc():
    nc = bacc.Bacc("TRN2", target_bir_lowering=False, debug=False)
    f32 = mybir.dt.float32
    fp16 = mybir.dt.float16
    fp8 = mybir.dt.float8e4

    enc_h = nc.dram_tensor("enc", [P, NT * D], fp8, kind="ExternalInput")
    dect_h = nc.dram_tensor("dect", [D, T], fp16, kind="ExternalInput")
    out_h = nc.dram_tensor("out", [D, T], fp16, kind="ExternalOutput")

    enc_v = enc_h.ap().rearrange("p (n d) -> p n d", d=D)
    dect_v = dect_h.ap()
    out_v = out_h.ap()

    with ExitStack() as ctx:
        tc = ctx.enter_context(tile.TileContext(nc))
        singles = ctx.enter_context(tc.tile_pool(name="singles", bufs=1))
        psum = ctx.enter_context(tc.tile_pool(name="psum", bufs=4, space="PSUM"))
        gpsum = ctx.enter_context(tc.tile_pool(name="gpsum", bufs=2, space="PSUM"))

        enc_sb = singles.tile([P, NT, D], fp8)
        dect_sb = singles.tile([P, T], fp16)
        g_sb = singles.tile([P, P], fp16)
        out_sb = singles.tile([P, T], fp16)

        warm = singles.tile([P, 128], fp16)
        wps = gpsum.tile([P, 512], f32, tag="warm")
        if W_WARM or W_FILL:
            nc.gpsimd.memset(warm[:], 0.0)
        for _ in range(W_WARM):
            nc.tensor.matmul(wps[:, :128], lhsT=warm[:], rhs=warm[:])

        # ---- loads ----
        # First issue on each ring carries half of enc (the Gram gates
        # everything); second issue on each ring carries half of dect.
        hnt = NT // 2
        nc.sync.dma_start(out=enc_sb[:, :hnt, :], in_=enc_v[:, :hnt, :])
        nc.scalar.dma_start(out=enc_sb[:, hnt:, :], in_=enc_v[:, hnt:, :])
        half = T // 2
        nc.sync.dma_start(out=dect_sb[:, :half], in_=dect_v[:, :half])
        nc.scalar.dma_start(out=dect_sb[:, half:], in_=dect_v[:, half:])

        # ---- Gram: G = sum_i enc_i^T enc_i (fp8 x fp8 -> f32 PSUM) ----
        g_ps = gpsum.tile([P, P], f32, tag="g")
        npair = NT // 2
        for j in range(npair):
            pair = enc_sb[:, 2 * j : 2 * j + 2, :]
            nc.tensor.matmul(
                g_ps[:],
                lhsT=pair,
                rhs=pair,
                start=(j == 0),
                stop=(j == npair - 1),
                perf_mode=mybir.MatmulPerfMode.DoubleRow,
            )
        # fillers keep the PE clock from resetting while DVE casts G
        for _ in range(W_FILL):
            nc.tensor.matmul(wps[:, :64], lhsT=warm[:], rhs=warm[:, :64])
        nc.vector.tensor_copy(g_sb[:], g_ps[:])

        # ---- OutT = G @ DecT: stationary G, 4 moving chunks ----
        # Each chunk's PSUM->SBUF copy is split across two engines
        # (COPY_SPLIT) so the store can issue ~half a copy earlier.
        n_final = T // FINAL_N
        hw = FINAL_N // 2
        for c in range(n_final):
            lo = c * FINAL_N
            op = psum.tile([P, FINAL_N], f32, tag="op")
            nc.tensor.matmul(op[:], lhsT=g_sb[:], rhs=dect_sb[:, lo : lo + FINAL_N])
            if COPY_SPLIT == "act":
                nc.vector.tensor_copy(out_sb[:, lo : lo + hw], op[:, :hw])
                nc.scalar.copy(out_sb[:, lo + hw : lo + FINAL_N], op[:, hw:])
            elif ACT_COPY and c % 2 == (1 if not SWAP_LANES else 0):
                nc.scalar.copy(out_sb[:, lo : lo + FINAL_N], op[:])
            else:
                nc.vector.tensor_copy(out_sb[:, lo : lo + FINAL_N], op[:])
            # with SWAP_LANES the LAST store rides the sync ring, whose
            # DGE delay is 650ns vs the scalar ring's 784ns.
            even_eng = nc.sync if not SWAP_LANES else nc.scalar
            odd_eng = nc.scalar if not SWAP_LANES else nc.sync
            deng = even_eng if c % 2 == 0 else odd_eng
            deng.dma_start(
                out=out_v[:, lo : lo + FINAL_N], in_=out_sb[:, lo : lo + FINAL_N]
            )

    nc.compile()
    return nc


_NC = []


def _get_nc():
    if not _NC:
        _NC.append(_build_nc())
    return _NC[0]


def _run(enc, dec, **kwargs):
    import ml_dtypes

    nc = _get_nc()
    f8 = ml_dtypes.float8_e4m3
    in_maps = []
    for b in range(B):
        in_maps.append(
            {
                "enc": np.ascontiguousarray(
                    enc[b]
                    .astype(f8)
                    .reshape(NT, P, D)
                    .transpose(1, 0, 2)
                    .reshape(P, NT * D)
                ),
                "dect": np.ascontiguousarray(dec[b].T.astype(np.float16)),
            }
        )
    res = run_bass_kernel_spmd(nc, in_maps, core_ids=list(range(B)), **kwargs)
    out = np.stack([res.results[b]["out"].T.astype(np.float32) for b in range(B)], axis=0)
    return np.ascontiguousarray(out), res


def kernel(encoder_hidden_states, decoder_hidden_states):
    enc = np.ascontiguousarray(np.asarray(encoder_hidden_states, dtype=np.float32))
    dec = np.ascontiguousarray(np.asarray(decoder_hidden_states, dtype=np.float32))
    assert enc.shape == (B, T, D) and dec.shape == (B, T, D)
    out, _ = _run(enc, dec)
    return out


# revision 12
# speedup vs baseline: 1.1379x; 1.1379x over previous
"""Trainium2 Bass kernel for nn_LuongAttention.

Reference math (per batch b):
    S   = Dec @ Enc^T          # [T_dec, T_enc]
    Out = S @ Enc              # [T_dec, D]

By associativity:  Out = Dec @ (Enc^T @ Enc) = Dec @ G with G = Enc^T Enc
a [D, D] = [128, 128] Gram matrix.  This removes the [2048, 2048]
intermediate entirely and makes the kernel memory-bound.

Sharding: data-parallel over batch B=8 -> one batch per NeuronCore.

Precision plan (tolerance 2e-2, measured 7.2e-3):
  - enc is fed as fp8 E4M3 (values ~N(0,1), |max| ~5 << 240): halves the
    encoder DMA bytes and enables DoubleRow matmuls (2 row-tiles per
    pass, 0.5 cycles/row - clock-state independent); quantization error
    averages out over the 2048-term Gram contraction.
  - dect stays fp16 (its error multiplies the ~2048 Gram diagonal);
    G cast to fp16 (fp8 G overflows); out fp16, host upcasts.

Schedule (from the deterministic cost model: dma issue ~650ns/ring,
DGE delay 650/784ns, transfers 360GB/s >=512B lines serialized on one
global DMA device, +900ns sem after each transfer; PE: 1.2GHz until
3us of *continuous* execution, then 2.4GHz):
  - sync ring: enc (256KB fp8), dect lo half; scalar ring: dect hi
    half (its descriptors arrive first -> transfers right after enc).
  - Warmup/filler matmuls keep the PE queue busy across the
    load-wait and the G-cast: a PE that goes idle pays a ~0.3-0.9us
    wake-up stall on its next matmul (measured), and continuous
    activity occasionally wins the run-random 1.2->2.4GHz clock flip.
  - Finals 4x512 cols; PSUM->SBUF copies alternate DVE / Scalar-ACT;
    stores alternate sync/scalar rings.
"""

import os
import sys
from contextlib import ExitStack

import numpy as np

for _p in (
    "/opt/trn_rl_repo",
    "/root/.axon_site",
    "/root/.axon_site/_ro/trn_rl_repo",
    "/root/.axon_site/_ro/pypackages",
):
    if os.path.isdir(_p) and _p not in sys.path:
        sys.path.append(_p)

import concourse.bacc as bacc
import concourse.mybir as mybir
import concourse.tile as tile
from concourse.bass_utils import run_bass_kernel_spmd

B, T, D, P = 8, 2048, 128, 128
NT = T // P  # 16 row tiles of 128

# tunables
FINAL_N = 512  # final matmul chunk (= PSUM bank)
W_WARM = 22  # 128-col warmups: keep PE busy until the enc sem (avoids ~0.3-0.9us PE wake-up stall at gram start; occasionally wins the 2.4GHz clock flip)
W_FILL = 10  # 64-col fillers bridging gram -> first final (avoids ~0.9us PE wake-up stall after the G cast; measured -0.5us)
ACT_COPY = True  # alternate PSUM->SBUF copies between DVE and ACT
COPY_SPLIT = ""  # "act" | "": split each copy across 2 engines (measured slower)
SWAP_LANES = True  # last store on sync ring (faster DGE), first copy on ACT


def _build_nc():
    nc = bacc.Bacc("TRN2", target_bir_lowering=False, debug=False)
    f32 = mybir.dt.float32
    fp16 = mybir.dt.float16
    fp8 = mybir.dt.float8e4

    enc_h = nc.dram_tensor("enc", [P, NT * D], fp8, kind="ExternalInput")
    dect_h = nc.dram_tensor("dect", [D, T], fp16, kind="ExternalInput")
    out_h = nc.dram_tensor("out", [D, T], fp16, kind="ExternalOutput")

    enc_v = enc_h.ap().rearrange("p (n d) -> p n d", d=D)
    dect_v = dect_h.ap()
    out_v = out_h.ap()

    with ExitStack() as ctx:
        tc = ctx.enter_context(tile.TileContext(nc))
        singles = ctx.enter_context(tc.tile_pool(name="singles", bufs=1))
        psum = ctx.enter_context(tc.tile_pool(name="psum", bufs=4, space="PSUM"))
        gpsum = ctx.enter_context(tc.tile_pool(name="gpsum", bufs=2, space="PSUM"))

        enc_sb = singles.tile([P, NT, D], fp8)
        dect_sb = singles.tile([P, T], fp16)
        g_sb = singles.tile([P, P], fp16)
        out_sb = singles.tile([P, T], fp16)

        warm = singles.tile([P, 128], fp16)
        wps = gpsum.tile([P, 512], f32, tag="warm")
        if W_WARM or W_FILL:
            nc.gpsimd.memset(warm[:], 0.0)
        for _ in range(W_WARM):
            nc.tensor.matmul(wps[:, :128], lhsT=warm[:], rhs=warm[:])

        # ---- loads ----
        # First issue on each ring carries half of enc (the Gram gates
        # everything); second issue on each ring carries half of dect.
        hnt = NT // 2
        nc.sync.dma_start(out=enc_sb[:, :hnt, :], in_=enc_v[:, :hnt, :])
        nc.scalar.dma_start(out=enc_sb[:, hnt:, :], in_=enc_v[:, hnt:, :])
        # dect in 4x128KB chunks alternating rings: DMA completion sems
        # stall ~0.9us on the two laggard SDMA engines for 256KB transfers
        # (NTFF sem track: incs 1-14 fast, 15/16 +0.93us) but not for
        # 128KB ones; smaller chunks cut the laggard tail off the f0 gate.
        qc = T // 4
        for k in range(4):
            eng = nc.sync if k % 2 == 0 else nc.scalar
            eng.dma_start(
                out=dect_sb[:, k * qc : (k + 1) * qc],
                in_=dect_v[:, k * qc : (k + 1) * qc],
            )

        # ---- Gram: G = sum_i enc_i^T enc_i (fp8 x fp8 -> f32 PSUM) ----
        g_ps = gpsum.tile([P, P], f32, tag="g")
        npair = NT // 2
        for j in range(npair):
            pair = enc_sb[:, 2 * j : 2 * j + 2, :]
            nc.tensor.matmul(
                g_ps[:],
                lhsT=pair,
                rhs=pair,
                start=(j == 0),
                stop=(j == npair - 1),
                perf_mode=mybir.MatmulPerfMode.DoubleRow,
            )
        # fillers keep the PE clock from resetting while DVE casts G
        for _ in range(W_FILL):
            nc.tensor.matmul(wps[:, :64], lhsT=warm[:], rhs=warm[:, :64])
        nc.vector.tensor_copy(g_sb[:], g_ps[:])

        # ---- OutT = G @ DecT: stationary G, 4 moving chunks ----
        # Each chunk's PSUM->SBUF copy is split across two engines
        # (COPY_SPLIT) so the store can issue ~half a copy earlier.
        n_final = T // FINAL_N
        hw = FINAL_N // 2
        for c in range(n_final):
            lo = c * FINAL_N
            op = psum.tile([P, FINAL_N], f32, tag="op")
            nc.tensor.matmul(op[:], lhsT=g_sb[:], rhs=dect_sb[:, lo : lo + FINAL_N])
            if COPY_SPLIT == "act":
                nc.vector.tensor_copy(out_sb[:, lo : lo + hw], op[:, :hw])
                nc.scalar.copy(out_sb[:, lo + hw : lo + FINAL_N], op[:, hw:])
            elif ACT_COPY and c % 2 == (1 if not SWAP_LANES else 0):
                nc.scalar.copy(out_sb[:, lo : lo + FINAL_N], op[:])
            else:
                nc.vector.tensor_copy(out_sb[:, lo : lo + FINAL_N], op[:])
            # with SWAP_LANES the LAST store rides the sync ring, whose
            # DGE delay is 650ns vs the scalar ring's 784ns.
            even_eng = nc.sync if not SWAP_LANES else nc.scalar
            odd_eng = nc.scalar if not SWAP_LANES else nc.sync
            deng = even_eng if c % 2 == 0 else odd_eng
            deng.dma_start(
                out=out_v[:, lo : lo + FINAL_N], in_=out_sb[:, lo : lo + FINAL_N]
            )

    nc.compile()
    return nc


_NC = []


def _get_nc():
    if not _NC:
        _NC.append(_build_nc())
    return _NC[0]


def _run(enc, dec, **kwargs):
    import ml_dtypes

    nc = _get_nc()
    f8 = ml_dtypes.float8_e4m3
    in_maps = []
    for b in range(B):
        in_maps.append(
            {
                "enc": np.ascontiguousarray(
                    enc[b]
                    .astype(f8)
                    .reshape(NT, P, D)
                    .transpose(1, 0, 2)
                    .reshape(P, NT * D)
                ),
                "dect": np.ascontiguousarray(dec[b].T.astype(np.float16)),
            }
        )
    res = run_bass_kernel_spmd(nc, in_maps, core_ids=list(range(B)), **kwargs)
    out = np.stack([res.results[b]["out"].T.astype(np.float32) for b in range(B)], axis=0)
    return np.ascontiguousarray(out), res


def kernel(encoder_hidden_states, decoder_hidden_states):
    enc = np.ascontiguousarray(np.asarray(encoder_hidden_states, dtype=np.float32))
    dec = np.ascontiguousarray(np.asarray(decoder_hidden_states, dtype=np.float32))
    assert enc.shape == (B, T, D) and dec.shape == (B, T, D)
    out, _ = _run(enc, dec)
    return out
